# revision 36
# baseline (speedup 1.0000x reference)
"""IterNorm (decorrelated batch norm) Trainium2 kernel.

Strategy (8 NeuronCores, data-parallel over N):
  - Each core gets 8 of 64 batch elements, kept resident in SBUF f32
    (4 tiles of [128, 2*3136] = 100KB/partition).
  - Pass 1 (per 128-col chunk): fp32r PE transpose -> cast-copy
    PSUM->SBUF bf16 (alternating ACT/DVE) into a [128,129] tile whose
    last column is memset to 1.0 (GpSimd), then one bf16 matmul
    accumulates [S | s] = Xt.T @ [Xt | 1] in PSUM (raw second moment +
    column sums in one accumulation group).
  - Cross-core reduction WITHOUT ncfw AllReduce: each core
    remote-DMA-broadcasts its [C,129] partial into per-source slots of
    every peer's SBUF (XOR-relative routing, descriptors generated
    during pass 1, fired by one trigger_dma after the kernel-entry
    barrier), then tree-reduces the 8 slots locally on DVE.
  - Stats + Newton-Schulz replicated per core.  Normalizing Sigma by
    C/tr (mean eigenvalue ~= 1) instead of 1/tr makes NS converge
    quadratically from the start: 4 total steps reach ~1e-6 of
    Sigma^{-1/2} (the same fixed point the reference's 10 trace-
    normalized steps approach).  NS matmuls run fp32r with a fused rhs
    [P | Sh]; all operands are symmetric so lhsT never needs a
    transpose.
  - Pass 2: out = wm @ x + (beta - wm @ mu) as fp32r matmuls (N=448 ->
    1 cy/row) straight from the resident f32 X, bias-add on ACT/DVE,
    DMAed out in half-batch (0.8MB) pieces.

kernel(**inputs) takes the FULL inputs and returns the FULL output.
"""

import sys

for _p in ("/opt/trn_rl_repo",):
    if _p not in sys.path:
        sys.path.insert(0, _p)

import numpy as np

C = 128
EPS = 1e-5
T_NS = 3  # total Newton-Schulz steps (first one folded analytically)
N_CORES = 8

FULL_N = 64
FULL_HW = 56 * 56  # 3136
NB = FULL_N // N_CORES  # batches per core = 8
TR = 128  # transpose / covariance chunk width
OC = 448  # pass-2 output chunk width (3136 = 7*448)


def build_program(nb=NB, hw=FULL_HW, n_cores=N_CORES, oc=OC):
    """Build + compile the Bass program. Returns (nc, meta)."""
    import concourse.bacc as bacc
    import concourse.tile as tile
    from concourse import mybir

    f32 = mybir.dt.float32
    f32r = mybir.dt.float32r
    bf16 = mybir.dt.bfloat16
    X = mybir.AxisListType.X
    AOT = mybir.AluOpType
    AFT = mybir.ActivationFunctionType

    assert nb % 2 == 0
    w2 = 2 * hw
    ntile = nb // 2
    assert w2 % TR == 0
    nchunk_t = w2 // TR
    ntr = ntile * nchunk_t
    assert hw % oc == 0
    noc_b = hw // oc
    m_tot = n_cores * nb * hw
    inv_m = 1.0 / float(m_tot)
    CW = C + 1  # comm width: [S | s]

    nc = bacc.Bacc("TRN2", target_bir_lowering=False, debug=False,
                   num_devices=n_cores)

    x_d = nc.dram_tensor("x", [nb, C, hw], f32, kind="ExternalInput")
    ident_d = nc.dram_tensor("ident", [C, C], f32, kind="ExternalInput")
    onesrow_d = nc.dram_tensor("onesrow", [1, C], f32, kind="ExternalInput")
    beta_d = nc.dram_tensor("beta", [C, 1], f32, kind="ExternalInput")
    out_d = nc.dram_tensor("out", [nb, C, hw], f32, kind="ExternalOutput")

    rdall = [(0, k) for k in range(8)]  # all 8 same-device peers, XOR-relative

    with tile.TileContext(nc, num_cores=n_cores) as tc:
        rsem = nc.alloc_semaphore("rdma_rx")  # data arrivals: 8 sources * 2
        hsem = nc.alloc_semaphore("rdma_hello")  # entry handshake: 8 * 2
        lsem = nc.alloc_semaphore("rdma_tx")  # local send completion: 2 * 16
        pid = nc.gpsimd.partition_id()
        with (
            tc.tile_pool(name="xres", bufs=1) as xpool,
            tc.tile_pool(name="consts", bufs=1) as consts,
            tc.tile_pool(name="stats", bufs=1) as stats,
            tc.tile_pool(name="nsp", bufs=2) as nsp,
            tc.tile_pool(name="warm", bufs=1, space="PSUM") as warmp,
        ):
            # ---- resident X tiles first (SP HWDGE ring) ----
            xt = []
            for t in range(ntile):
                xtile = xpool.tile([C, w2], f32r, tag=f"x{t}")
                for h in range(2):
                    if t == 0 and h == 0:
                        q4 = hw // 4
                        for p in range(4):
                            nc.sync.dma_start(
                                out=xtile[:, p * q4:(p + 1) * q4],
                                in_=x_d[0][:, p * q4:(p + 1) * q4]
                                .bitcast(f32r))
                    else:
                        nc.sync.dma_start(
                            out=xtile[:, h * hw:(h + 1) * hw],
                            in_=x_d[2 * t + h].bitcast(f32r),
                        )
                xt.append(xtile)

            # ---- consts on the ACT HWDGE ring (independent queue) ----
            ident = consts.tile([C, C], f32, tag="ident")
            nc.scalar.dma_start(out=ident, in_=ident_d[:, :])
            identr = consts.tile([C, C], f32r, tag="identr")
            nc.scalar.dma_start(out=identr, in_=ident_d[:, :].bitcast(f32r))
            beta_sb = consts.tile([C, 1], f32, tag="beta")
            nc.scalar.dma_start(out=beta_sb, in_=beta_d[:, :])
            ones_cc = consts.tile([C, C], f32, tag="ones_cc")
            nc.vector.memset(ones_cc, 1.0)

            # warm the ACT LUTs (Sqrt, Identity) off the critical path
            scr_a = stats.tile([C, 1], f32, tag="scr_a")
            scr_b = stats.tile([C, 1], f32, tag="scr_b")
            nc.vector.memset(scr_a, 1.0)
            nc.scalar.sqrt(scr_b, scr_a)
            nc.scalar.activation(scr_b, scr_a, AFT.Identity, bias=scr_a,
                                 scale=1.0)

            # Dummy-matmul filler for the exchange gap (keeps the PE clock
            # gate up so Newton-Schulz runs at full rate).
            warm_in = stats.tile([C, 256], bf16, tag="warm_in")
            nc.vector.memset(warm_in, 0.0)
            warm_ps = warmp.tile([C, 256], f32, tag="warm_ps")

            def warm_fill(n):
                for _ in range(n):
                    nc.tensor.matmul(warm_ps, lhsT=warm_in[:, 0:C],
                                     rhs=warm_in, start=True, stop=True,
                                     skip_group_check=True)



            # comm: this core's [S | s] partial; gather: all 8 cores'
            comm = stats.tile([C, CW], f32, tag="comm")
            gather = stats.tile([C, n_cores * CW], f32, tag="gather")
            junk = stats.tile([C, 1], f32, tag="junk")
            junkc = stats.tile([C, 1], f32, tag="junkc")

            # Exchange descriptors (prepare-only; comm is read when
            # trigger_dma fires, so descgen runs during pass 1).  ONE
            # broadcast with all 8 dests real: every core sends comm to all
            # peers, writing slot <own rank> — so each receiver's slot j
            # holds rank j's partial.  All-real keeps every lane's 64
            # descriptors useful (a mostly-None dest list pads dummy
            # descriptors per line, which cost the same ~0.1us each).
            for j in tc.Switch(pid, n_cores):
                nc.gpsimd.remote_dma_broadcast(
                    out_ap=gather[:, j * CW:(j + 1) * CW],
                    in_ap=comm[:, :],
                    remote_sem=rsem,
                    local_sem=lsem,
                    rdests=rdall,
                )

            # ---- pass 1: [S | s] via dual accumulators ----
            # Two 128-col transposes land side by side in one PSUM tile so
            # the PSUM->SBUF bf16 cast is ONE [128,256] copy per pair (the
            # copies, not the PE, are the pass-1 throughput limit).  The ts
            # layout is [1 | a | b | 1]: the a-matmul accumulates [s_a|S_a]
            # into bank A, the b-matmul [S_b|s_b] into bank B, so both rhs
            # slices stay contiguous and the ones columns still yield the
            # column sums; the two banks merge with two DVE adds at the end.
            assert ntr % 2 == 0
            ndc = ntr // 2
            with (
                tc.tile_pool(name="tps", bufs=5, space="PSUM") as psumT,
                tc.tile_pool(name="tss", bufs=8) as sbufT,
                tc.tile_pool(name="accs", bufs=1, space="PSUM") as psumS,
            ):
                Sa_ps = psumS.tile([C, C + 1], f32, tag="Sa")
                Sb_ps = psumS.tile([C, C + 1], f32, tag="Sb")

                tp_pend = {}

                def emit_tr(dc):
                    tp = psumT.tile([C, 2 * TR], f32, tag="tp")
                    for h in range(2):
                        c = 2 * dc + h
                        t, j = divmod(c, nchunk_t)
                        nc.tensor.transpose(
                            tp[:, h * TR:(h + 1) * TR].bitcast(f32r),
                            xt[t][:, TR * j:TR * (j + 1)],
                            identr)
                    tp_pend[dc] = tp

                def emit_acc(dc):
                    tp = tp_pend.pop(dc)
                    ts = sbufT.tile([C, 2 * TR + 2], bf16, tag="ts")
                    # ~60/40 DVE/ACT split keeps both copy engines loaded
                    if dc % 5 < 2:
                        nc.scalar.copy(ts[:, 1:2 * TR + 1], tp)
                    else:
                        nc.vector.tensor_copy(ts[:, 1:2 * TR + 1], tp)
                    nc.gpsimd.memset(ts[:, 0:1], 1.0)
                    nc.gpsimd.memset(ts[:, 2 * TR + 1:2 * TR + 2], 1.0)
                    nc.tensor.matmul(
                        Sa_ps, lhsT=ts[:, 1:TR + 1], rhs=ts[:, 0:TR + 1],
                        start=(dc == 0), stop=(dc == ndc - 1),
                        skip_group_check=True)
                    nc.tensor.matmul(
                        Sb_ps, lhsT=ts[:, TR + 1:2 * TR + 1],
                        rhs=ts[:, TR + 1:2 * TR + 2],
                        start=(dc == 0), stop=(dc == ndc - 1),
                        skip_group_check=True)

                LOOK = 4
                for dc in range(min(LOOK, ndc)):
                    emit_tr(dc)
                for dc in range(ndc):
                    if dc + LOOK < ndc:
                        emit_tr(dc + LOOK)
                    emit_acc(dc)

                # merge: S = Sa[:,1:] + Sb[:,:C]; s = Sa[:,0] + Sb[:,C]
                # (one op may read only one PSUM input: stage Sb via SBUF)
                sb_sb = stats.tile([C, C + 1], f32, tag="sb_sb")
                nc.scalar.copy(sb_sb, Sb_ps)
                nc.vector.tensor_tensor(comm[:, 0:C], Sa_ps[:, 1:C + 1],
                                        sb_sb[:, 0:C], AOT.add)
                nc.vector.tensor_tensor(comm[:, C:C + 1], Sa_ps[:, 0:1],
                                        sb_sb[:, C:C + 1], AOT.add)

            # Keep the PE clock gate at 8/8 across the exchange gap so the
            # Newton-Schulz matmuls run at full rate.  These have no deps:
            # they execute right after the last S matmul, during the wait.
            warm_fill(14)

            # ---- cross-core exchange + local tree reduce ----
            # The critical section's entry gate waits for every data dep of
            # its body (incl. comm, via the anchor copy), so the trigger
            # cannot fire before comm is written.  Within the section each
            # engine runs FIFO: gpsimd waits for all peers' hellos + its own
            # descgen, then fires the one prepared broadcast; DVE waits for
            # all 8 arrivals, then tree-reduces in place.
            with tc.tile_critical():
                nc.vector.tensor_copy(junkc, comm[:, 0:1])
                nc.gpsimd.bir_kernel_barrier_wait([list(range(n_cores))])
                nc.gpsimd.trigger_dma(count=1)
                nc.vector.wait_ge(rsem, 2 * n_cores)
                g = [gather[:, k * CW:(k + 1) * CW] for k in range(n_cores)]
                for a, b in ((0, 1), (2, 3), (4, 5), (6, 7),
                             (0, 2), (4, 6), (0, 4)):
                    nc.vector.tensor_tensor(g[a], g[a], g[b], AOT.add)
            red = gather[:, 0:CW]

            # ---- stats -> Sigma -> Newton-Schulz -> wm, bias ----
            with tc.tile_pool(name="psb", bufs=1, space="PSUM") as psumB:
                # chain A: mu -> muT -> outer -> Sig -> Sig2
                mu = stats.tile([C, 1], f32, tag="mu")
                nc.scalar.mul(mu, red[:, C:C + 1], inv_m)
                muT_ps = psumB.tile([1, C], f32, tag="muT")
                nc.tensor.matmul(muT_ps, lhsT=mu, rhs=ident, start=True,
                                 stop=True)
                muT = stats.tile([1, C], f32, tag="muTs")
                nc.scalar.copy(muT, muT_ps)
                outer_ps = psumB.tile([C, C], f32, tag="outer")
                nc.tensor.matmul(outer_ps, lhsT=muT, rhs=muT, start=True,
                                 stop=True)
                # Sigma = S/m - mu mu^T + eps I
                Sig = stats.tile([C, C], f32, tag="Sig")
                nc.vector.scalar_tensor_tensor(
                    Sig, in0=red[:, 0:C], scalar=inv_m, in1=outer_ps,
                    op0=AOT.mult, op1=AOT.subtract)
                Sig2 = stats.tile([C, C], f32, tag="Sig2")
                nc.vector.scalar_tensor_tensor(
                    Sig2, in0=ident, scalar=EPS, in1=Sig,
                    op0=AOT.mult, op1=AOT.add)

                # chain B (parallel): tr(Sigma) = tr(S)/m - |mu|^2 + C eps,
                # computed at 128 partitions throughout; the ones_cc matmul
                # both sums over partitions and broadcasts the result.
                dm = stats.tile([C, C], f32, tag="dm")
                nc.vector.tensor_tensor(dm, red[:, 0:C], ident, AOT.mult)
                dvec = stats.tile([C, 1], f32, tag="dvec")
                nc.vector.reduce_sum(dvec, dm, axis=X)
                musq_el = stats.tile([C, 1], f32, tag="musq_el")
                nc.vector.tensor_tensor(musq_el, mu, mu, AOT.mult)
                w_el = stats.tile([C, 1], f32, tag="w_el")
                nc.vector.scalar_tensor_tensor(
                    w_el, in0=dvec, scalar=inv_m, in1=musq_el,
                    op0=AOT.mult, op1=AOT.subtract)
                trb_ps = psumB.tile([C, 1], f32, tag="trb")
                nc.tensor.matmul(trb_ps, lhsT=ones_cc, rhs=w_el, start=True,
                                 stop=True)
                # trn = tr(Sigma) / C  (mean eigenvalue, ~= 1), broadcast
                trn = stats.tile([C, 1], f32, tag="trn")
                nc.vector.tensor_scalar(trn, in0=trb_ps,
                                        scalar1=float(C) * EPS,
                                        scalar2=1.0 / float(C),
                                        op0=AOT.add, op1=AOT.mult)
                rb = stats.tile([C, 1], f32, tag="rbs")
                nc.vector.reciprocal(rb, trn)  # c = C / tr(Sigma)
                srb = stats.tile([C, 1], f32, tag="srb")
                nc.scalar.sqrt(srb, rb)

                # NS state tile: [P | Sh], Sh = 0.5 * Sigma * c
                Pfull = stats.tile([C, 2 * C], f32r, tag="P0")
                nc.vector.tensor_scalar(Pfull[:, C:2 * C], in0=Sig2,
                                        scalar1=rb, scalar2=0.5,
                                        op0=AOT.mult, op1=AOT.mult)
                # P1 = 1.5 I - Sh  (P0 = I folded in)
                nc.vector.scalar_tensor_tensor(
                    Pfull[:, 0:C], in0=ident, scalar=1.5,
                    in1=Pfull[:, C:2 * C], op0=AOT.mult, op1=AOT.subtract)
                for it in range(T_NS - 1):
                    P = Pfull[:, 0:C]
                    Sh = Pfull[:, C:2 * C]
                    Pt = nsp.tile([C, C], f32, tag="Pt")
                    nc.scalar.mul(Pt, P, 1.5)
                    # [A | D] = P @ [P | Sh]   (fp32r, N=256)
                    AD_ps = psumB.tile([C, 2 * C], f32, tag="AD")
                    nc.tensor.matmul(AD_ps, lhsT=P, rhs=Pfull,
                                     start=True, stop=True)
                    AD_sb = nsp.tile([C, 2 * C], f32r, tag="ADsb")
                    nc.scalar.copy(AD_sb[:, 0:C], AD_ps[:, 0:C])
                    nc.vector.tensor_copy(AD_sb[:, C:2 * C],
                                          AD_ps[:, C:2 * C])
                    # E = A @ D = P^3 Sh
                    E_ps = psumB.tile([C, C], f32, tag="E")
                    nc.tensor.matmul(
                        E_ps, lhsT=AD_sb[:, 0:C], rhs=AD_sb[:, C:2 * C],
                        start=True, stop=True)
                    Pn = nsp.tile([C, 2 * C], f32r, tag="Pn")
                    if it < T_NS - 2:
                        nc.vector.tensor_copy(Pn[:, C:2 * C], Sh)
                    nc.vector.tensor_tensor(Pn[:, 0:C], Pt, E_ps,
                                            AOT.subtract)
                    Pfull = Pn
                wm = stats.tile([C, C], f32r, tag="wm")
                nc.vector.tensor_scalar(wm, in0=Pfull[:, 0:C], scalar1=srb,
                                        scalar2=None, op0=AOT.mult)
                wm_f = stats.tile([C, C], f32, tag="wmf")
                nc.vector.tensor_scalar(wm_f, in0=Pfull[:, 0:C], scalar1=srb,
                                        scalar2=None, op0=AOT.mult)
                wmu_ps = psumB.tile([C, 1], f32, tag="wmu")
                nc.tensor.matmul(wmu_ps, lhsT=wm_f, rhs=mu, start=True,
                                 stop=True)
                bias = stats.tile([C, 1], f32, tag="bias")
                nc.vector.tensor_tensor(bias, beta_sb, wmu_ps, AOT.subtract)

            # ---- leave the exchange sems clean for the next execution ----
            # (Tile's exit drain only clears its own sems.)  The gpsimd read
            # of the reduce output anchors this section after the exchange.
            with tc.tile_critical():
                nc.gpsimd.tensor_copy(junk, gather[:, 0:1])
                nc.gpsimd.wait_ge(rsem, 2 * n_cores)
                nc.gpsimd.wait_ge(lsem, 16)
                nc.gpsimd.sem_clear(rsem)
                nc.gpsimd.sem_clear(hsem)
                nc.gpsimd.sem_clear(lsem)

            # ---- pass 2: out = wm @ x + bias (fp32r) ----
            with (
                tc.tile_pool(name="ops", bufs=4, space="PSUM") as psumO,
                tc.tile_pool(name="obuf", bufs=3) as opool,
            ):
                # split points for the per-batch output DMA (earlier start,
                # shorter tail); cut after chunk 3 of 7
                cut = (noc_b + 1) // 2 if noc_b > 1 else 0
                for t in range(ntile):
                    for h in range(2):
                        ob = opool.tile([C, hw], f32, tag="ob")
                        for q in range(noc_b):
                            lo = h * hw + oc * q
                            o_ps = psumO.tile([C, oc], f32, tag="ops")
                            nc.tensor.matmul(
                                o_ps, lhsT=wm, rhs=xt[t][:, lo:lo + oc],
                                start=True, stop=True)
                            dst = ob[:, oc * q:oc * (q + 1)]
                            if q % 2 == 0:
                                nc.scalar.activation(dst, o_ps, AFT.Identity,
                                                     bias=bias, scale=1.0)
                            else:
                                nc.vector.tensor_scalar(dst, in0=o_ps,
                                                        scalar1=bias,
                                                        scalar2=None,
                                                        op0=AOT.add)
                            if q == cut - 1:
                                nc.sync.dma_start(
                                    out=out_d[2 * t + h][:, 0:oc * cut],
                                    in_=ob[:, 0:oc * cut])
                        nc.sync.dma_start(
                            out=out_d[2 * t + h][:, oc * cut:hw],
                            in_=ob[:, oc * cut:hw])

    nc.compile()
    meta = dict(nb=nb, hw=hw, n_cores=n_cores)
    return nc, meta


def make_in_maps(X, beta, nb=NB, hw=FULL_HW, n_cores=N_CORES):
    """X: (n_cores*nb, C, hw) f32; beta: (C,). Returns per-core input dicts."""
    ident = np.eye(C, dtype=np.float32)
    onesrow = np.ones((1, C), dtype=np.float32)
    beta2 = np.asarray(beta, dtype=np.float32).reshape(C, 1)
    in_maps = []
    for k in range(n_cores):
        in_maps.append({
            "x": np.ascontiguousarray(X[k * nb:(k + 1) * nb]),
            "ident": ident,
            "onesrow": onesrow,
            "beta": beta2,
        })
    return in_maps


_CACHE = {}


def _get_program():
    if "nc" not in _CACHE:
        _CACHE["nc"] = build_program()
    return _CACHE["nc"]


def kernel(X, beta, running_mean, running_cov):
    """Full inputs in, full outputs out. running_* unused (they only feed
    the discarded running-stat outputs of the reference)."""
    from concourse import bass_utils

    X = np.asarray(X, dtype=np.float32)
    n, c, h, w = X.shape
    assert (n, c) == (FULL_N, C) and h * w == FULL_HW
    Xf = X.reshape(n, c, h * w)

    nc, meta = _get_program()
    in_maps = make_in_maps(Xf, beta)
    res = bass_utils.run_bass_kernel_spmd(nc, in_maps, list(range(N_CORES)))
    out = np.empty((n, c, h * w), dtype=np.float32)
    for k in range(N_CORES):
        out[k * NB:(k + 1) * NB] = res.results[k]["out"]
    return out.reshape(n, c, h, w)
